# revision 19
# baseline (speedup 1.0000x reference)
"""Two-layer GCN block on 8 Trainium2 NeuronCores (Bass/Tile).

out = GCNConv2(relu(GCNConv1(x, edge_index)))  with symmetric deg^-1/2 norm
and self-loops, matching PyG GCNConv defaults (b1 == 0 per the spec).

Strategy (memory-bound gather/segment-sum workload):
  - Nodes are sharded across 8 cores (12500 each, padded to 12544 = 98
    windows of 128 destinations). Edges live on the core owning their dst.
  - Linearity lets the layer-1 transform commute through aggregation:
        sum_s dinv_s (x W1)_s = (sum_s dinv_s x_s) W1
    so the device gathers rows of the pre-scaled table xs = dinv*x and
    applies W1 once per 128-dst window instead of once per node.
  - Per window: per-edge source rows fetched with dma_gather (int16 indices,
    node table split into 32768-row blocks), combined with a 0/1 selection
    matrix S (vector engine is_equal against an iota row) via PSUM-accumulated
    matmuls; the "swapped" operand order yields the transposed aggregate
    [feat, dst] so the whole window epilogue
        ts2 = dinv^2 * (relu((aggT)^T W1) W2)
    runs with zero on-chip transposes (dst-side dinv folds through relu and
    W2 because dinv > 0 and b1 = 0).
  - One AllGather exchanges the per-core ts2 shards (bf16).
  - Layer-2 aggregation gathers from the full ts2 and writes
    out = dinv * agg + b2 for the core's own windows.

The schedule (group counts per window and source block) is padded to the
per-(window, block) maximum across cores, so a single SPMD program serves all
8 cores; per-core variation lives entirely in input data (indices, rel-dst
values, degree columns).
"""

import os
import sys

if "/opt/trn_rl_repo" not in sys.path:
    sys.path.insert(0, "/opt/trn_rl_repo")
os.environ.setdefault("NEURON_SCRATCHPAD_PAGE_SIZE", "512")

from dataclasses import dataclass

import ml_dtypes
import numpy as np

P = 128


@dataclass(frozen=True)
class Cfg:
    n: int  # number of real nodes
    n_cores: int = 8
    scw: int = 7  # windows per superchunk
    blk: int = 32768  # gather table rows per int16-addressable block

    @property
    def npc(self):
        return self.n // self.n_cores

    @property
    def wpc(self):
        return -(-self.npc // P)

    @property
    def ppc(self):
        return self.wpc * P

    @property
    def npad(self):
        return self.ppc * self.n_cores

    @property
    def nwin(self):
        return self.wpc * self.n_cores

    @property
    def nsc(self):
        assert self.wpc % self.scw == 0, (self.wpc, self.scw)
        return self.wpc // self.scw

    @property
    def nb(self):
        return -(-self.npad // self.blk)


CFG = Cfg(n=100000)


# ----------------------------------------------------------------------------
# Host-side preprocessing: edge schedule shared by both layers.
# ----------------------------------------------------------------------------


def _preprocess(cfg: Cfg, edge_index: np.ndarray):
    n = cfg.n
    src = np.concatenate([edge_index[0], np.arange(n, dtype=np.int64)])
    dst = np.concatenate([edge_index[1], np.arange(n, dtype=np.int64)])
    deg = np.bincount(dst, minlength=n).astype(np.float32)

    def pad_id(v):
        c = v // cfg.npc
        return c * cfg.ppc + (v - c * cfg.npc)

    srcp = pad_id(src)
    dstp = pad_id(dst)
    core = dst // cfg.npc
    locw = (dstp % cfg.ppc) // P  # local window on owning core
    rel = (dstp % P).astype(np.float32)
    blk = srcp // cfg.blk
    lsrc = (srcp - blk * cfg.blk).astype(np.int16)

    key = (core * cfg.wpc + locw) * cfg.nb + blk
    counts = np.bincount(key, minlength=cfg.n_cores * cfg.wpc * cfg.nb).reshape(
        cfg.n_cores, cfg.wpc, cfg.nb
    )
    G = -(-counts // P)  # ceil
    G = G.max(axis=0)  # [wpc, nb] shared schedule
    gtot = int(G.sum())

    goff = np.zeros((cfg.wpc, cfg.nb), dtype=np.int64)
    np.cumsum(G.ravel()[:-1], out=goff.ravel()[1:])

    ni = np.zeros((cfg.nsc, cfg.nb), dtype=np.int64)
    for s in range(cfg.nsc):
        ni[s] = G[s * cfg.scw : (s + 1) * cfg.scw].sum(axis=0) * P
    icoff = np.zeros(cfg.nsc * cfg.nb, dtype=np.int64)
    np.cumsum(ni.ravel()[:-1] // 16, out=icoff[1:])
    icoff = icoff.reshape(cfg.nsc, cfg.nb)
    icols = int(ni.sum() // 16)

    boff = np.zeros((cfg.wpc, cfg.nb), dtype=np.int64)
    for s in range(cfg.nsc):
        js = slice(s * cfg.scw, (s + 1) * cfg.scw)
        boff[js] = np.cumsum(
            np.vstack([np.zeros(cfg.nb, np.int64), G[js][:-1] * P]), axis=0
        )

    rd_all = np.full((cfg.n_cores, P, gtot), -1.0, dtype=np.float32)
    idx_all = np.zeros((cfg.n_cores, P, icols), dtype=np.int16)

    order = np.lexsort((blk, locw, core))
    srt_core = core[order]
    srt_key = (locw * cfg.nb + blk)[order]
    srt_rel = rel[order]
    srt_lsrc = lsrc[order]

    core_bounds = np.searchsorted(srt_core, np.arange(cfg.n_cores + 1))
    for c in range(cfg.n_cores):
        lo, hi = core_bounds[c], core_bounds[c + 1]
        k = srt_key[lo:hi]
        r = srt_rel[lo:hi]
        ls = srt_lsrc[lo:hi]
        bucket_start = np.searchsorted(k, np.arange(cfg.wpc * cfg.nb))
        q = np.arange(k.size) - bucket_start[k]
        w = k // cfg.nb
        b = k % cfg.nb
        col = goff[w, b] + q // P
        rd_all[c, q % P, col] = r
        i = boff[w, b] + q
        s = w // cfg.scw
        idx_all[c, (i % 16), icoff[s, b] + i // 16] = ls
    idx_all = np.tile(idx_all[:, :16, :], (1, 8, 1))

    degp = np.ones(cfg.npad, dtype=np.float32)
    degp[pad_id(np.arange(n))] = deg
    deg_all = degp.reshape(cfg.nwin, P).T.copy()  # [128, nwin] window-major

    return {
        "G": G,
        "ni": ni,
        "icoff": icoff,
        "gtot": gtot,
        "icols": icols,
        "rd_all": rd_all,
        "idx_all": idx_all,
        "deg_all": deg_all,
        "degp": degp,
        "pad_id": pad_id,
    }


# ----------------------------------------------------------------------------
# Device program
# ----------------------------------------------------------------------------


def _build(
    cfg: Cfg,
    G: np.ndarray,
    ni: np.ndarray,
    icoff: np.ndarray,
    gtot: int,
    icols: int,
    no_collective: bool = False,
    reps: int = 1,
):
    import concourse.bacc as bacc
    import concourse.bass as bass
    import concourse.mybir as mybir
    import concourse.tile as tile

    f32 = mybir.dt.float32
    bf16 = mybir.dt.bfloat16
    i16 = mybir.dt.int16
    EQ = mybir.AluOpType.is_equal
    MUL = mybir.AluOpType.mult
    ADD = mybir.AluOpType.add
    ACT_COPY = mybir.ActivationFunctionType.Copy
    ACT_RELU = mybir.ActivationFunctionType.Relu

    nc = bacc.Bacc("TRN2", target_bir_lowering=False, debug=False)

    xs = nc.declare_dram_parameter("xs", [cfg.npad, P], bf16, isOutput=False)
    w1 = nc.declare_dram_parameter("w1", [P, P], bf16, isOutput=False)
    w2 = nc.declare_dram_parameter("w2", [P, P], bf16, isOutput=False)
    b2r = nc.declare_dram_parameter("b2r", [P, P], f32, isOutput=False)
    iota = nc.declare_dram_parameter("iota", [P, P], bf16, isOutput=False)
    deg_own = nc.declare_dram_parameter("deg_own", [P, cfg.wpc], f32, isOutput=False)
    idx_all = nc.declare_dram_parameter("idx_all", [P, icols], i16, isOutput=False)
    rd_all = nc.declare_dram_parameter("rd_all", [P, gtot], f32, isOutput=False)
    out = nc.declare_dram_parameter("out", [cfg.ppc, P], f32, isOutput=True)

    ts2s = nc.dram_tensor("ts2s", [cfg.ppc, P], bf16)
    ts2f = nc.dram_tensor("ts2f", [cfg.npad, P], bf16, addr_space="Shared")

    ts2s_r = ts2s.ap().rearrange("(t p) f -> p t f", p=P)
    out_r = out.ap().rearrange("(t p) f -> p t f", p=P)

    n_blk_rows = [min(cfg.blk, cfg.npad - b * cfg.blk) for b in range(cfg.nb)]

    with tile.TileContext(nc) as tc:
        cpool = tc.tile_pool(name="const", bufs=1)
        cp = cpool.__enter__()
        w1_t = cp.tile([P, P], bf16)
        nc.sync.dma_start(w1_t[:], w1[:, :])
        w2_t = cp.tile([P, P], bf16)
        nc.sync.dma_start(w2_t[:], w2[:, :])
        b2_t = cp.tile([P, P], f32)
        nc.sync.dma_start(b2_t[:], b2r[:, :])
        iota_t = cp.tile([P, P], bf16)
        nc.sync.dma_start(iota_t[:], iota[:, :])

        dego_t = cp.tile([P, cfg.wpc], f32)
        nc.sync.dma_start(dego_t[:], deg_own[:, :])
        rcpo_t = cp.tile([P, cfg.wpc], f32)
        nc.vector.reciprocal(rcpo_t[:], dego_t[:])
        dinv_own = cp.tile([P, cfg.wpc], f32)
        nc.scalar.activation(dinv_own[:], rcpo_t[:], mybir.ActivationFunctionType.Sqrt)
        dinv2_own = cp.tile([P, cfg.wpc], f32)
        nc.vector.tensor_tensor(
            out=dinv2_own[:], in0=dinv_own[:], in1=dinv_own[:], op=MUL
        )

        def aggregate(table, layer1: bool, rep: int = 0):
            sfx = ("1" if layer1 else "2") + (f"r{rep}" if rep else "")
            with (
                tc.tile_pool(name=f"pg_idx{sfx}", bufs=2) as p_idx,
                tc.tile_pool(name=f"pg_msg{sfx}", bufs=2) as p_msg,
                tc.tile_pool(name=f"pg_rd{sfx}", bufs=2) as p_rd,
                tc.tile_pool(name=f"pg_s{sfx}", bufs=6) as p_s,
                tc.tile_pool(name=f"pg_eo{sfx}", bufs=3) as p_eo,
                tc.tile_pool(name=f"pg_ps{sfx}", bufs=2, space="PSUM") as p_ps,
                tc.tile_pool(name=f"pg_ps{sfx}b", bufs=2, space="PSUM") as p_ps2,
            ):
                rdcol = 0
                for s in range(cfg.nsc):
                    js = slice(s * cfg.scw, (s + 1) * cfg.scw)
                    sc_cols = int(G[js].sum())
                    sc_rd0 = rdcol
                    rdt = p_rd.tile([P, max(1, sc_cols)], f32, tag="rdt")
                    if sc_cols:
                        nc.sync.dma_start(
                            rdt[:, :sc_cols], rd_all[:, rdcol : rdcol + sc_cols]
                        )
                    mg = []
                    for b in range(cfg.nb):
                        nib = int(ni[s, b])
                        mx = int(ni[:, b].max())
                        mt = p_msg.tile([P, max(P, mx)], bf16, tag=f"mg{b}")
                        if nib:
                            it = p_idx.tile(
                                [P, max(1, mx // 16)], i16, tag=f"it{b}"
                            )
                            nc.sync.dma_start(
                                it[:, : nib // 16],
                                idx_all[:, icoff[s, b] : icoff[s, b] + nib // 16],
                            )
                            nc.gpsimd.dma_gather(
                                out_ap=mt[:, :nib].rearrange("p (g e) -> p g e", e=P),
                                in_ap=table[b * cfg.blk : b * cfg.blk + n_blk_rows[b], :],
                                idxs_ap=it[:, : nib // 16],
                                num_idxs=nib,
                                num_idxs_reg=nib,
                                elem_size=P,
                                single_packet=False,
                            )
                        mg.append(mt)

                    bpos = [0] * cfg.nb
                    for j in range(s * cfg.scw, (s + 1) * cfg.scw):
                        gw = int(G[j].sum())
                        ps = p_ps.tile([P, P], f32, space="PSUM", tag="agg")
                        k = 0
                        for b in range(cfg.nb):
                            for _g in range(int(G[j, b])):
                                st = p_s.tile([P, P], bf16, tag="sel")
                                lc = rdcol - sc_rd0
                                nc.vector.tensor_scalar(
                                    out=st[:],
                                    in0=iota_t[:],
                                    scalar1=rdt[:, lc : lc + 1],
                                    scalar2=None,
                                    op0=EQ,
                                )
                                e0 = bpos[b]
                                if layer1:
                                    nc.tensor.matmul(
                                        ps[:],
                                        lhsT=mg[b][:, e0 : e0 + P],
                                        rhs=st[:],
                                        start=(k == 0),
                                        stop=(k == gw - 1),
                                    )
                                else:
                                    nc.tensor.matmul(
                                        ps[:],
                                        lhsT=st[:],
                                        rhs=mg[b][:, e0 : e0 + P],
                                        start=(k == 0),
                                        stop=(k == gw - 1),
                                    )
                                bpos[b] += P
                                rdcol += 1
                                k += 1
                        if gw == 0:
                            st = p_s.tile([P, P], bf16, tag="sel")
                            nc.vector.tensor_tensor(
                                out=st[:],
                                in0=iota_t[:],
                                in1=iota_t[:],
                                op=mybir.AluOpType.subtract,
                            )
                            nc.tensor.matmul(
                                ps[:], lhsT=st[:], rhs=st[:], start=True, stop=True
                            )
                        if layer1:
                            # ps = agg1^T [feat_in, dst]
                            c1 = p_eo.tile([P, P], bf16, tag="c1")
                            nc.scalar.activation(c1[:], ps[:], ACT_COPY)
                            ps2 = p_ps2.tile([P, P], f32, space="PSUM", tag="t1")
                            nc.tensor.matmul(
                                ps2[:], lhsT=w1_t[:], rhs=c1[:], start=True, stop=True
                            )
                            # ps2 = (agg1 W1)^T [feat_h, dst]
                            rt = p_eo.tile([P, P], bf16, tag="rt")
                            nc.scalar.activation(rt[:], ps2[:], ACT_RELU)
                            ps3 = p_ps.tile([P, P], f32, space="PSUM", tag="mm2")
                            nc.tensor.matmul(
                                ps3[:], lhsT=rt[:], rhs=w2_t[:], start=True, stop=True
                            )
                            # ts2 = dinv^2 * (relu(agg1 W1) W2)  [dst, feat2]
                            t2 = p_eo.tile([P, P], bf16, tag="t2")
                            nc.scalar.activation(
                                t2[:], ps3[:], ACT_COPY, scale=dinv2_own[:, j : j + 1]
                            )
                            nc.sync.dma_start(ts2s_r[:, j, :], t2[:])
                        else:
                            o1 = p_eo.tile([P, P], f32, tag="o1")
                            nc.scalar.activation(
                                o1[:], ps[:], ACT_COPY, scale=dinv_own[:, j : j + 1]
                            )
                            o2 = p_eo.tile([P, P], f32, tag="o2")
                            nc.vector.tensor_tensor(
                                out=o2[:], in0=o1[:], in1=b2_t[:], op=ADD
                            )
                            nc.sync.dma_start(out_r[:, j, :], o2[:])

        for rep in range(reps):
            # ---- phase B: layer-1 aggregation (gathers from xs) + ts2 ------
            aggregate(xs[:, :], layer1=True, rep=rep)

            # ---- exchange --------------------------------------------------
            if not no_collective:
                nc.gpsimd.collective_compute(
                    "AllGather",
                    mybir.AluOpType.bypass,
                    replica_groups=[list(range(cfg.n_cores))],
                    ins=[ts2s[:, :]],
                    outs=[ts2f[:, :]],
                )

            # ---- phase C: layer-2 aggregation + output ---------------------
            aggregate(xs[:, :] if no_collective else ts2f.ap(), layer1=False, rep=rep)

        cpool.__exit__(None, None, None)

    nc.compile()
    return nc


# ----------------------------------------------------------------------------
# Entry point
# ----------------------------------------------------------------------------

_CACHE = {}


def _prep_inputs(cfg: Cfg, pre, x, W1, W2, b2):
    n = cfg.n
    dinv = 1.0 / np.sqrt(pre["degp"])  # padded slots have deg=1
    xsp = np.zeros((cfg.npad, P), dtype=np.float32)
    xsp[pre["pad_id"](np.arange(n))] = np.asarray(x, np.float32)
    xsp *= dinv[:, None]
    xs = xsp.astype(ml_dtypes.bfloat16)

    iota = np.broadcast_to(np.arange(P, dtype=np.float32), (P, P)).astype(
        ml_dtypes.bfloat16
    )
    in_maps = []
    for c in range(cfg.n_cores):
        in_maps.append(
            {
                "xs": xs,
                "w1": np.asarray(W1, np.float32).astype(ml_dtypes.bfloat16),
                "w2": np.asarray(W2, np.float32).astype(ml_dtypes.bfloat16),
                "b2r": np.broadcast_to(np.asarray(b2, np.float32), (P, P)).copy(),
                "iota": np.ascontiguousarray(iota),
                "deg_own": pre["deg_all"][:, c * cfg.wpc : (c + 1) * cfg.wpc],
                "idx_all": pre["idx_all"][c],
                "rd_all": np.ascontiguousarray(pre["rd_all"][c]),
            }
        )
    return in_maps


def _get_nc(cfg: Cfg, pre):
    key = (cfg, pre["gtot"], pre["icols"], pre["G"].tobytes(), pre["ni"].tobytes())
    if key not in _CACHE:
        _CACHE[key] = _build(
            cfg, pre["G"], pre["ni"], pre["icoff"], pre["gtot"], pre["icols"]
        )
    return _CACHE[key]


def _kernel_impl(cfg: Cfg, x, edge_index, W1, b1, W2, b2):
    from concourse.bass_utils import run_bass_kernel_spmd

    assert np.allclose(b1, 0.0), "kernel assumes b1 == 0 (spec fill: zeros)"

    pre = _preprocess(cfg, np.asarray(edge_index, dtype=np.int64))
    nc = _get_nc(cfg, pre)
    in_maps = _prep_inputs(cfg, pre, x, W1, W2, b2)

    res = run_bass_kernel_spmd(nc, in_maps, list(range(cfg.n_cores)))
    parts = [res.results[c]["out"][: cfg.npc] for c in range(cfg.n_cores)]
    return np.concatenate(parts, axis=0)


def kernel(x, edge_index, W1, b1, W2, b2):
    return _kernel_impl(CFG, x, edge_index, W1, b1, W2, b2)


# ----------------------------------------------------------------------------
# Steady-state timing support (no NTFF profiling under this axon client: we
# time repeated executions with device-resident inputs and subtract the
# dispatch floor measured with a null kernel).
# ----------------------------------------------------------------------------


def _make_runner(nc, n_cores):
    import jax
    from jax.sharding import Mesh, NamedSharding, PartitionSpec
    from jax.experimental.shard_map import shard_map

    from concourse import bass2jax, mybir

    bass2jax.install_neuronx_cc_hook()
    partition_name = nc.partition_id_tensor.name if nc.partition_id_tensor else None
    in_names, out_names, out_avals, zero_outs = [], [], [], []
    for alloc in nc.m.functions[0].allocations:
        if not isinstance(alloc, mybir.MemoryLocationSet):
            continue
        name = alloc.memorylocations[0].name
        if alloc.kind == "ExternalInput":
            if name != partition_name:
                in_names.append(name)
        elif alloc.kind == "ExternalOutput":
            shape = tuple(alloc.tensor_shape)
            dtype = mybir.dt.np(alloc.dtype)
            out_names.append(name)
            out_avals.append(jax.core.ShapedArray(shape, dtype))
            zero_outs.append(np.zeros(shape, dtype))
    n_params = len(in_names)
    all_in_names = list(in_names) + list(out_names)
    if partition_name is not None:
        all_in_names.append(partition_name)

    def _body(*args):
        operands = list(args)
        if partition_name is not None:
            operands.append(bass2jax.partition_id_tensor())
        outs = bass2jax._bass_exec_p.bind(
            *operands,
            out_avals=tuple(out_avals),
            in_names=tuple(all_in_names),
            out_names=tuple(out_names),
            lowering_input_output_aliases=(),
            sim_require_finite=True,
            sim_require_nnan=True,
            nc=nc,
        )
        return tuple(outs)

    devices = jax.devices()[:n_cores]
    mesh = Mesh(np.asarray(devices), ("core",))
    in_specs = (PartitionSpec("core"),) * (n_params + len(out_names))
    out_specs = (PartitionSpec("core"),) * len(out_names)
    fn = jax.jit(
        shard_map(
            _body, mesh=mesh, in_specs=in_specs, out_specs=out_specs, check_rep=False
        ),
        keep_unused=True,
    )
    sharding = NamedSharding(mesh, PartitionSpec("core"))

    def run(in_maps, iters=1):
        import time as _t

        concat = [
            np.concatenate([np.asarray(in_maps[c][n]) for c in range(n_cores)], axis=0)
            for n in in_names
        ]
        concat += [
            np.zeros((n_cores * z.shape[0], *z.shape[1:]), z.dtype) for z in zero_outs
        ]
        dev_in = [jax.device_put(a, sharding) for a in concat]
        outs = fn(*dev_in)
        jax.block_until_ready(outs)
        times = []
        for _ in range(iters):
            t0 = _t.perf_counter()
            outs = fn(*dev_in)
            jax.block_until_ready(outs)
            times.append(_t.perf_counter() - t0)
        return outs, out_names, out_avals, times

    return run


def time_kernel(x, edge_index, W1, b1, W2, b2, iters=30, reps=9):
    cfg = CFG
    pre = _preprocess(cfg, np.asarray(edge_index, dtype=np.int64))
    in_maps = _prep_inputs(cfg, pre, x, W1, W2, b2)

    nc1 = _get_nc(cfg, pre)
    run1 = _make_runner(nc1, cfg.n_cores)
    _, _, _, t1 = run1(in_maps, iters=iters)

    ncR = _build(
        cfg, pre["G"], pre["ni"], pre["icoff"], pre["gtot"], pre["icols"], reps=reps
    )
    runR = _make_runner(ncR, cfg.n_cores)
    _, _, _, tR = runR(in_maps, iters=iters)

    est = (min(tR) - min(t1)) / (reps - 1)
    print(
        f"(x1: min {min(t1)*1e3:.3f} med {sorted(t1)[len(t1)//2]*1e3:.3f} ms; "
        f"x{reps}: min {min(tR)*1e3:.3f} med {sorted(tR)[len(tR)//2]*1e3:.3f} ms)"
    )
    return est * 1e9


# revision 20
# speedup vs baseline: 1.2071x; 1.2071x over previous
"""Two-layer GCN block on 8 Trainium2 NeuronCores (Bass/Tile).

out = GCNConv2(relu(GCNConv1(x, edge_index)))  with symmetric deg^-1/2 norm
and self-loops, matching PyG GCNConv defaults (b1 == 0 per the spec).

Strategy (memory-bound gather/segment-sum workload):
  - Nodes are sharded across 8 cores (12500 each, padded to 12544 = 98
    windows of 128 destinations). Edges live on the core owning their dst.
  - Linearity lets the layer-1 transform commute through aggregation:
        sum_s dinv_s (x W1)_s = (sum_s dinv_s x_s) W1
    so the device gathers rows of the pre-scaled table xs = dinv*x and
    applies W1 once per 128-dst window instead of once per node.
  - Per window: per-edge source rows fetched with dma_gather (int16 indices,
    node table split into 32768-row blocks), combined with a 0/1 selection
    matrix S (vector engine is_equal against an iota row) via PSUM-accumulated
    matmuls; the "swapped" operand order yields the transposed aggregate
    [feat, dst] so the whole window epilogue
        ts2 = dinv^2 * (relu((aggT)^T W1) W2)
    runs with zero on-chip transposes (dst-side dinv folds through relu and
    W2 because dinv > 0 and b1 = 0).
  - One AllGather exchanges the per-core ts2 shards (bf16).
  - Layer-2 aggregation gathers from the full ts2 and writes
    out = dinv * agg + b2 for the core's own windows.

The schedule (group counts per window and source block) is padded to the
per-(window, block) maximum across cores, so a single SPMD program serves all
8 cores; per-core variation lives entirely in input data (indices, rel-dst
values, degree columns).
"""

import os
import sys

if "/opt/trn_rl_repo" not in sys.path:
    sys.path.insert(0, "/opt/trn_rl_repo")
os.environ.setdefault("NEURON_SCRATCHPAD_PAGE_SIZE", "512")

from dataclasses import dataclass

import ml_dtypes
import numpy as np

P = 128


@dataclass(frozen=True)
class Cfg:
    n: int  # number of real nodes
    n_cores: int = 8
    scw: int = 7  # windows per superchunk
    blk: int = 32768  # gather table rows per int16-addressable block

    @property
    def npc(self):
        return self.n // self.n_cores

    @property
    def wpc(self):
        return -(-self.npc // P)

    @property
    def ppc(self):
        return self.wpc * P

    @property
    def npad(self):
        return self.ppc * self.n_cores

    @property
    def nwin(self):
        return self.wpc * self.n_cores

    @property
    def nsc(self):
        assert self.wpc % self.scw == 0, (self.wpc, self.scw)
        return self.wpc // self.scw

    @property
    def nb(self):
        return -(-self.npad // self.blk)


CFG = Cfg(n=100000)


# ----------------------------------------------------------------------------
# Host-side preprocessing: edge schedule shared by both layers.
# ----------------------------------------------------------------------------


def _preprocess(cfg: Cfg, edge_index: np.ndarray):
    n = cfg.n
    src = np.concatenate([edge_index[0], np.arange(n, dtype=np.int64)])
    dst = np.concatenate([edge_index[1], np.arange(n, dtype=np.int64)])
    deg = np.bincount(dst, minlength=n).astype(np.float32)

    def pad_id(v):
        c = v // cfg.npc
        return c * cfg.ppc + (v - c * cfg.npc)

    srcp = pad_id(src)
    dstp = pad_id(dst)
    core = dst // cfg.npc
    locw = (dstp % cfg.ppc) // P  # local window on owning core
    rel = (dstp % P).astype(np.float32)
    blk = srcp // cfg.blk
    lsrc = (srcp - blk * cfg.blk).astype(np.int16)

    key = (core * cfg.wpc + locw) * cfg.nb + blk
    counts = np.bincount(key, minlength=cfg.n_cores * cfg.wpc * cfg.nb).reshape(
        cfg.n_cores, cfg.wpc, cfg.nb
    )
    G = -(-counts // P)  # ceil
    G = G.max(axis=0)  # [wpc, nb] shared schedule
    gtot = int(G.sum())

    goff = np.zeros((cfg.wpc, cfg.nb), dtype=np.int64)
    np.cumsum(G.ravel()[:-1], out=goff.ravel()[1:])

    ni = np.zeros((cfg.nsc, cfg.nb), dtype=np.int64)
    for s in range(cfg.nsc):
        ni[s] = G[s * cfg.scw : (s + 1) * cfg.scw].sum(axis=0) * P
    icoff = np.zeros(cfg.nsc * cfg.nb, dtype=np.int64)
    np.cumsum(ni.ravel()[:-1] // 16, out=icoff[1:])
    icoff = icoff.reshape(cfg.nsc, cfg.nb)
    icols = int(ni.sum() // 16)

    boff = np.zeros((cfg.wpc, cfg.nb), dtype=np.int64)
    for s in range(cfg.nsc):
        js = slice(s * cfg.scw, (s + 1) * cfg.scw)
        boff[js] = np.cumsum(
            np.vstack([np.zeros(cfg.nb, np.int64), G[js][:-1] * P]), axis=0
        )

    rd_all = np.full((cfg.n_cores, P, gtot), -1.0, dtype=np.float32)
    idx_all = np.zeros((cfg.n_cores, P, icols), dtype=np.int16)

    order = np.lexsort((lsrc, blk, locw, core))
    srt_core = core[order]
    srt_key = (locw * cfg.nb + blk)[order]
    srt_rel = rel[order]
    srt_lsrc = lsrc[order]

    core_bounds = np.searchsorted(srt_core, np.arange(cfg.n_cores + 1))
    for c in range(cfg.n_cores):
        lo, hi = core_bounds[c], core_bounds[c + 1]
        k = srt_key[lo:hi]
        r = srt_rel[lo:hi]
        ls = srt_lsrc[lo:hi]
        bucket_start = np.searchsorted(k, np.arange(cfg.wpc * cfg.nb))
        q = np.arange(k.size) - bucket_start[k]
        w = k // cfg.nb
        b = k % cfg.nb
        col = goff[w, b] + q // P
        rd_all[c, q % P, col] = r
        i = boff[w, b] + q
        s = w // cfg.scw
        idx_all[c, (i % 16), icoff[s, b] + i // 16] = ls
    idx_all = np.tile(idx_all[:, :16, :], (1, 8, 1))

    degp = np.ones(cfg.npad, dtype=np.float32)
    degp[pad_id(np.arange(n))] = deg
    deg_all = degp.reshape(cfg.nwin, P).T.copy()  # [128, nwin] window-major

    return {
        "G": G,
        "ni": ni,
        "icoff": icoff,
        "gtot": gtot,
        "icols": icols,
        "rd_all": rd_all,
        "idx_all": idx_all,
        "deg_all": deg_all,
        "degp": degp,
        "pad_id": pad_id,
    }


# ----------------------------------------------------------------------------
# Device program
# ----------------------------------------------------------------------------


def _build(
    cfg: Cfg,
    G: np.ndarray,
    ni: np.ndarray,
    icoff: np.ndarray,
    gtot: int,
    icols: int,
    no_collective: bool = False,
    reps: int = 1,
):
    import concourse.bacc as bacc
    import concourse.bass as bass
    import concourse.mybir as mybir
    import concourse.tile as tile

    f32 = mybir.dt.float32
    bf16 = mybir.dt.bfloat16
    i16 = mybir.dt.int16
    EQ = mybir.AluOpType.is_equal
    MUL = mybir.AluOpType.mult
    ADD = mybir.AluOpType.add
    ACT_COPY = mybir.ActivationFunctionType.Copy
    ACT_RELU = mybir.ActivationFunctionType.Relu

    nc = bacc.Bacc("TRN2", target_bir_lowering=False, debug=False, num_swdge_queues=2)

    xs = nc.declare_dram_parameter("xs", [cfg.npad, P], bf16, isOutput=False)
    w1 = nc.declare_dram_parameter("w1", [P, P], bf16, isOutput=False)
    w2 = nc.declare_dram_parameter("w2", [P, P], bf16, isOutput=False)
    b2r = nc.declare_dram_parameter("b2r", [P, P], f32, isOutput=False)
    iota = nc.declare_dram_parameter("iota", [P, P], bf16, isOutput=False)
    deg_own = nc.declare_dram_parameter("deg_own", [P, cfg.wpc], f32, isOutput=False)
    idx_all = nc.declare_dram_parameter("idx_all", [P, icols], i16, isOutput=False)
    rd_all = nc.declare_dram_parameter("rd_all", [P, gtot], f32, isOutput=False)
    out = nc.declare_dram_parameter("out", [cfg.ppc, P], f32, isOutput=True)

    ts2s = nc.dram_tensor("ts2s", [cfg.ppc, P], bf16)
    ts2f = nc.dram_tensor("ts2f", [cfg.npad, P], bf16, addr_space="Shared")

    ts2s_r = ts2s.ap().rearrange("(t p) f -> p t f", p=P)
    out_r = out.ap().rearrange("(t p) f -> p t f", p=P)

    n_blk_rows = [min(cfg.blk, cfg.npad - b * cfg.blk) for b in range(cfg.nb)]

    with tile.TileContext(nc) as tc:
        cpool = tc.tile_pool(name="const", bufs=1)
        cp = cpool.__enter__()
        w1_t = cp.tile([P, P], bf16)
        nc.sync.dma_start(w1_t[:], w1[:, :])
        w2_t = cp.tile([P, P], bf16)
        nc.sync.dma_start(w2_t[:], w2[:, :])
        b2_t = cp.tile([P, P], f32)
        nc.sync.dma_start(b2_t[:], b2r[:, :])
        iota_t = cp.tile([P, P], bf16)
        nc.sync.dma_start(iota_t[:], iota[:, :])

        dego_t = cp.tile([P, cfg.wpc], f32)
        nc.sync.dma_start(dego_t[:], deg_own[:, :])
        rcpo_t = cp.tile([P, cfg.wpc], f32)
        nc.vector.reciprocal(rcpo_t[:], dego_t[:])
        dinv_own = cp.tile([P, cfg.wpc], f32)
        nc.scalar.activation(dinv_own[:], rcpo_t[:], mybir.ActivationFunctionType.Sqrt)
        dinv2_own = cp.tile([P, cfg.wpc], f32)
        nc.vector.tensor_tensor(
            out=dinv2_own[:], in0=dinv_own[:], in1=dinv_own[:], op=MUL
        )

        def aggregate(table, layer1: bool, rep: int = 0):
            sfx = ("1" if layer1 else "2") + (f"r{rep}" if rep else "")
            with (
                tc.tile_pool(name=f"pg_idx{sfx}", bufs=2) as p_idx,
                tc.tile_pool(name=f"pg_msg{sfx}", bufs=2) as p_msg,
                tc.tile_pool(name=f"pg_rd{sfx}", bufs=2) as p_rd,
                tc.tile_pool(name=f"pg_s{sfx}", bufs=6) as p_s,
                tc.tile_pool(name=f"pg_eo{sfx}", bufs=3) as p_eo,
                tc.tile_pool(name=f"pg_ps{sfx}", bufs=2, space="PSUM") as p_ps,
                tc.tile_pool(name=f"pg_ps{sfx}b", bufs=2, space="PSUM") as p_ps2,
            ):
                rdcol = 0
                for s in range(cfg.nsc):
                    js = slice(s * cfg.scw, (s + 1) * cfg.scw)
                    sc_cols = int(G[js].sum())
                    sc_rd0 = rdcol
                    rdt = p_rd.tile([P, max(1, sc_cols)], f32, tag="rdt")
                    if sc_cols:
                        nc.sync.dma_start(
                            rdt[:, :sc_cols], rd_all[:, rdcol : rdcol + sc_cols]
                        )
                    mg = []
                    for b in range(cfg.nb):
                        nib = int(ni[s, b])
                        mx = int(ni[:, b].max())
                        mt = p_msg.tile([P, max(P, mx)], bf16, tag=f"mg{b}")
                        if nib:
                            it = p_idx.tile(
                                [P, max(1, mx // 16)], i16, tag=f"it{b}"
                            )
                            nc.sync.dma_start(
                                it[:, : nib // 16],
                                idx_all[:, icoff[s, b] : icoff[s, b] + nib // 16],
                            )
                            nc.gpsimd.dma_gather(
                                out_ap=mt[:, :nib].rearrange("p (g e) -> p g e", e=P),
                                in_ap=table[b * cfg.blk : b * cfg.blk + n_blk_rows[b], :],
                                idxs_ap=it[:, : nib // 16],
                                num_idxs=nib,
                                num_idxs_reg=nib,
                                elem_size=P,
                                single_packet=False,
                                queue_num=(s * cfg.nb + b) % 2,
                            )
                        mg.append(mt)

                    bpos = [0] * cfg.nb
                    for j in range(s * cfg.scw, (s + 1) * cfg.scw):
                        gw = int(G[j].sum())
                        ps = p_ps.tile([P, P], f32, space="PSUM", tag="agg")
                        k = 0
                        for b in range(cfg.nb):
                            for _g in range(int(G[j, b])):
                                st = p_s.tile([P, P], bf16, tag="sel")
                                lc = rdcol - sc_rd0
                                nc.vector.tensor_scalar(
                                    out=st[:],
                                    in0=iota_t[:],
                                    scalar1=rdt[:, lc : lc + 1],
                                    scalar2=None,
                                    op0=EQ,
                                )
                                e0 = bpos[b]
                                if layer1:
                                    nc.tensor.matmul(
                                        ps[:],
                                        lhsT=mg[b][:, e0 : e0 + P],
                                        rhs=st[:],
                                        start=(k == 0),
                                        stop=(k == gw - 1),
                                    )
                                else:
                                    nc.tensor.matmul(
                                        ps[:],
                                        lhsT=st[:],
                                        rhs=mg[b][:, e0 : e0 + P],
                                        start=(k == 0),
                                        stop=(k == gw - 1),
                                    )
                                bpos[b] += P
                                rdcol += 1
                                k += 1
                        if gw == 0:
                            st = p_s.tile([P, P], bf16, tag="sel")
                            nc.vector.tensor_tensor(
                                out=st[:],
                                in0=iota_t[:],
                                in1=iota_t[:],
                                op=mybir.AluOpType.subtract,
                            )
                            nc.tensor.matmul(
                                ps[:], lhsT=st[:], rhs=st[:], start=True, stop=True
                            )
                        if layer1:
                            # ps = agg1^T [feat_in, dst]
                            c1 = p_eo.tile([P, P], bf16, tag="c1")
                            nc.scalar.activation(c1[:], ps[:], ACT_COPY)
                            ps2 = p_ps2.tile([P, P], f32, space="PSUM", tag="t1")
                            nc.tensor.matmul(
                                ps2[:], lhsT=w1_t[:], rhs=c1[:], start=True, stop=True
                            )
                            # ps2 = (agg1 W1)^T [feat_h, dst]
                            rt = p_eo.tile([P, P], bf16, tag="rt")
                            nc.scalar.activation(rt[:], ps2[:], ACT_RELU)
                            ps3 = p_ps.tile([P, P], f32, space="PSUM", tag="mm2")
                            nc.tensor.matmul(
                                ps3[:], lhsT=rt[:], rhs=w2_t[:], start=True, stop=True
                            )
                            # ts2 = dinv^2 * (relu(agg1 W1) W2)  [dst, feat2]
                            t2 = p_eo.tile([P, P], bf16, tag="t2")
                            nc.scalar.activation(
                                t2[:], ps3[:], ACT_COPY, scale=dinv2_own[:, j : j + 1]
                            )
                            nc.sync.dma_start(ts2s_r[:, j, :], t2[:])
                        else:
                            o1 = p_eo.tile([P, P], f32, tag="o1")
                            nc.scalar.activation(
                                o1[:], ps[:], ACT_COPY, scale=dinv_own[:, j : j + 1]
                            )
                            o2 = p_eo.tile([P, P], f32, tag="o2")
                            nc.vector.tensor_tensor(
                                out=o2[:], in0=o1[:], in1=b2_t[:], op=ADD
                            )
                            nc.sync.dma_start(out_r[:, j, :], o2[:])

        for rep in range(reps):
            # ---- phase B: layer-1 aggregation (gathers from xs) + ts2 ------
            aggregate(xs[:, :], layer1=True, rep=rep)

            # ---- exchange --------------------------------------------------
            if not no_collective:
                nc.gpsimd.collective_compute(
                    "AllGather",
                    mybir.AluOpType.bypass,
                    replica_groups=[list(range(cfg.n_cores))],
                    ins=[ts2s[:, :]],
                    outs=[ts2f[:, :]],
                )

            # ---- phase C: layer-2 aggregation + output ---------------------
            aggregate(xs[:, :] if no_collective else ts2f.ap(), layer1=False, rep=rep)

        cpool.__exit__(None, None, None)

    nc.compile()
    return nc


# ----------------------------------------------------------------------------
# Entry point
# ----------------------------------------------------------------------------

_CACHE = {}


def _prep_inputs(cfg: Cfg, pre, x, W1, W2, b2):
    n = cfg.n
    dinv = 1.0 / np.sqrt(pre["degp"])  # padded slots have deg=1
    xsp = np.zeros((cfg.npad, P), dtype=np.float32)
    xsp[pre["pad_id"](np.arange(n))] = np.asarray(x, np.float32)
    xsp *= dinv[:, None]
    xs = xsp.astype(ml_dtypes.bfloat16)

    iota = np.broadcast_to(np.arange(P, dtype=np.float32), (P, P)).astype(
        ml_dtypes.bfloat16
    )
    in_maps = []
    for c in range(cfg.n_cores):
        in_maps.append(
            {
                "xs": xs,
                "w1": np.asarray(W1, np.float32).astype(ml_dtypes.bfloat16),
                "w2": np.asarray(W2, np.float32).astype(ml_dtypes.bfloat16),
                "b2r": np.broadcast_to(np.asarray(b2, np.float32), (P, P)).copy(),
                "iota": np.ascontiguousarray(iota),
                "deg_own": pre["deg_all"][:, c * cfg.wpc : (c + 1) * cfg.wpc],
                "idx_all": pre["idx_all"][c],
                "rd_all": np.ascontiguousarray(pre["rd_all"][c]),
            }
        )
    return in_maps


def _get_nc(cfg: Cfg, pre):
    key = (cfg, pre["gtot"], pre["icols"], pre["G"].tobytes(), pre["ni"].tobytes())
    if key not in _CACHE:
        _CACHE[key] = _build(
            cfg, pre["G"], pre["ni"], pre["icoff"], pre["gtot"], pre["icols"]
        )
    return _CACHE[key]


def _kernel_impl(cfg: Cfg, x, edge_index, W1, b1, W2, b2):
    from concourse.bass_utils import run_bass_kernel_spmd

    assert np.allclose(b1, 0.0), "kernel assumes b1 == 0 (spec fill: zeros)"

    pre = _preprocess(cfg, np.asarray(edge_index, dtype=np.int64))
    nc = _get_nc(cfg, pre)
    in_maps = _prep_inputs(cfg, pre, x, W1, W2, b2)

    res = run_bass_kernel_spmd(nc, in_maps, list(range(cfg.n_cores)))
    parts = [res.results[c]["out"][: cfg.npc] for c in range(cfg.n_cores)]
    return np.concatenate(parts, axis=0)


def kernel(x, edge_index, W1, b1, W2, b2):
    return _kernel_impl(CFG, x, edge_index, W1, b1, W2, b2)


# ----------------------------------------------------------------------------
# Steady-state timing support (no NTFF profiling under this axon client: we
# time repeated executions with device-resident inputs and subtract the
# dispatch floor measured with a null kernel).
# ----------------------------------------------------------------------------


def _make_runner(nc, n_cores):
    import jax
    from jax.sharding import Mesh, NamedSharding, PartitionSpec
    from jax.experimental.shard_map import shard_map

    from concourse import bass2jax, mybir

    bass2jax.install_neuronx_cc_hook()
    partition_name = nc.partition_id_tensor.name if nc.partition_id_tensor else None
    in_names, out_names, out_avals, zero_outs = [], [], [], []
    for alloc in nc.m.functions[0].allocations:
        if not isinstance(alloc, mybir.MemoryLocationSet):
            continue
        name = alloc.memorylocations[0].name
        if alloc.kind == "ExternalInput":
            if name != partition_name:
                in_names.append(name)
        elif alloc.kind == "ExternalOutput":
            shape = tuple(alloc.tensor_shape)
            dtype = mybir.dt.np(alloc.dtype)
            out_names.append(name)
            out_avals.append(jax.core.ShapedArray(shape, dtype))
            zero_outs.append(np.zeros(shape, dtype))
    n_params = len(in_names)
    all_in_names = list(in_names) + list(out_names)
    if partition_name is not None:
        all_in_names.append(partition_name)

    def _body(*args):
        operands = list(args)
        if partition_name is not None:
            operands.append(bass2jax.partition_id_tensor())
        outs = bass2jax._bass_exec_p.bind(
            *operands,
            out_avals=tuple(out_avals),
            in_names=tuple(all_in_names),
            out_names=tuple(out_names),
            lowering_input_output_aliases=(),
            sim_require_finite=True,
            sim_require_nnan=True,
            nc=nc,
        )
        return tuple(outs)

    devices = jax.devices()[:n_cores]
    mesh = Mesh(np.asarray(devices), ("core",))
    in_specs = (PartitionSpec("core"),) * (n_params + len(out_names))
    out_specs = (PartitionSpec("core"),) * len(out_names)
    fn = jax.jit(
        shard_map(
            _body, mesh=mesh, in_specs=in_specs, out_specs=out_specs, check_rep=False
        ),
        keep_unused=True,
    )
    sharding = NamedSharding(mesh, PartitionSpec("core"))

    def run(in_maps, iters=1):
        import time as _t

        concat = [
            np.concatenate([np.asarray(in_maps[c][n]) for c in range(n_cores)], axis=0)
            for n in in_names
        ]
        concat += [
            np.zeros((n_cores * z.shape[0], *z.shape[1:]), z.dtype) for z in zero_outs
        ]
        dev_in = [jax.device_put(a, sharding) for a in concat]
        outs = fn(*dev_in)
        jax.block_until_ready(outs)
        times = []
        for _ in range(iters):
            t0 = _t.perf_counter()
            outs = fn(*dev_in)
            jax.block_until_ready(outs)
            times.append(_t.perf_counter() - t0)
        return outs, out_names, out_avals, times

    return run


def time_kernel(x, edge_index, W1, b1, W2, b2, iters=30, reps=9):
    cfg = CFG
    pre = _preprocess(cfg, np.asarray(edge_index, dtype=np.int64))
    in_maps = _prep_inputs(cfg, pre, x, W1, W2, b2)

    nc1 = _get_nc(cfg, pre)
    run1 = _make_runner(nc1, cfg.n_cores)
    _, _, _, t1 = run1(in_maps, iters=iters)

    ncR = _build(
        cfg, pre["G"], pre["ni"], pre["icoff"], pre["gtot"], pre["icols"], reps=reps
    )
    runR = _make_runner(ncR, cfg.n_cores)
    _, _, _, tR = runR(in_maps, iters=iters)

    est = (min(tR) - min(t1)) / (reps - 1)
    print(
        f"(x1: min {min(t1)*1e3:.3f} med {sorted(t1)[len(t1)//2]*1e3:.3f} ms; "
        f"x{reps}: min {min(tR)*1e3:.3f} med {sorted(tR)[len(tR)//2]*1e3:.3f} ms)"
    )
    return est * 1e9
